# revision 6
# baseline (speedup 1.0000x reference)
"""Chamfer loss kernel for Trainium2 (8 NeuronCores, batch-parallel).

Problem: preds [8, 4096, 3] f32, gts [8, 4096, 3] f32.
  P[b,n,m] = ||gts[b,n] - preds[b,m]||^2  (expanded form)
  loss = sum_{b,m} min_n P[b,n,m] + sum_{b,n} min_m P[b,n,m]

Sharding: one batch per NeuronCore (data parallel over B=8).

Device algorithm (per core, one batch):
  Host augments points to 5-dim vectors so a single K=5 matmul emits
  squared distances directly into PSUM:
      a_n = [-2*x_n, ||x_n||^2, 1]   (x = gts row)
      b_m = [ y_m,   1, ||y_m||^2]   (y = preds row)
      a_n . b_m = ||x_n - y_m||^2  (same expanded form as the reference)
  Pass A: rows = n (gts) on partitions -> row-min over m -> loss_2 terms.
  Pass B: rows = m (preds) on partitions -> row-min over n -> loss_1 terms.
  VectorE min-reduces each [128, 2048] PSUM tile along the free axis,
  then folds row mins into per-partition sums [128, 1].
Host sums the 8x128 partial sums (the gather/unshard step).
"""

import sys

import numpy as np

sys.path.insert(0, "/opt/trn_rl_repo")

B = 8
N = 4096  # points per cloud (both preds and gts)
D5 = 5  # augmented dim
P = 128  # partitions
CHUNK = 2048  # psum tile free width (4 banks)
N_CORES = 8


def _build_kernel_body(ctx, tc, out_ap, ab_ap):
    import concourse.bass as bass
    from concourse import mybir

    nc = tc.nc
    f32 = mybir.dt.float32
    n_rowtiles = N // P  # 32
    n_chunks = N // CHUNK  # 2

    const = ctx.enter_context(tc.tile_pool(name="const", bufs=1))
    psum = ctx.enter_context(tc.tile_pool(name="psum", bufs=2, space="PSUM"))

    # One DMA for both operand matrices -> a single DMA semaphore for the
    # first matmuls to wait on (the Matmult/LDWEIGHTS struct has only one
    # sync-wait slot in walrus codegen).
    ab_sb = const.tile([D5, 2 * N], f32)
    nc.sync.dma_start(out=ab_sb[:], in_=ab_ap[:])
    at_sb = ab_sb[:, :N]
    bt_sb = ab_sb[:, N:]

    # rowmins[:, pass*32+i, h] = min over chunk h's columns of row-tile i
    rowmins = const.tile([P, 2 * n_rowtiles, n_chunks], f32)

    for pass_idx, (lhs_src, rhs_src) in enumerate(((at_sb, bt_sb), (bt_sb, at_sb))):
        for i in range(n_rowtiles):
            lhsT = lhs_src[:, i * P : (i + 1) * P]  # [5, 128]
            for h in range(n_chunks):
                ps = psum.tile([P, CHUNK], f32, tag="ps")
                for j in range(CHUNK // 512):
                    rhs = rhs_src[:, h * CHUNK + j * 512 : h * CHUNK + (j + 1) * 512]
                    nc.tensor.matmul(
                        ps[:, j * 512 : (j + 1) * 512],
                        lhsT,
                        rhs,
                        start=True,
                        stop=True,
                    )
                nc.vector.tensor_reduce(
                    out=rowmins[:, pass_idx * n_rowtiles + i, h : h + 1],
                    in_=ps[:],
                    axis=mybir.AxisListType.X,
                    op=mybir.AluOpType.min,
                )

    # Combine the per-chunk mins: true row min = min over the chunk axis.
    comb = const.tile([P, 2 * n_rowtiles], f32)
    nc.vector.tensor_tensor(
        out=comb[:],
        in0=rowmins[:, :, 0],
        in1=rowmins[:, :, 1],
        op=mybir.AluOpType.min,
    )
    # Per-partition sum of all row mins (both passes).
    sums = const.tile([P, 1], f32)
    nc.vector.tensor_reduce(
        out=sums[:],
        in_=comb[:],
        axis=mybir.AxisListType.X,
        op=mybir.AluOpType.add,
    )
    nc.sync.dma_start(out=out_ap[:], in_=sums[:])


def _build_nc():
    from contextlib import ExitStack

    import concourse.bass as bass
    import concourse.tile as tile
    from concourse import mybir

    nc = bass.Bass("TRN2", target_bir_lowering=False, debug=False)
    f32 = mybir.dt.float32
    ab = nc.dram_tensor("ab", [D5, 2 * N], f32, kind="ExternalInput").ap()
    out = nc.dram_tensor("out", [P, 1], f32, kind="ExternalOutput").ap()
    with tile.TileContext(nc) as tc, ExitStack() as ctx:
        _build_kernel_body(ctx, tc, out, ab)
    _fix_sync_waits(nc)
    return nc


def _fix_sync_waits(nc):
    """Work around walrus's one-sync-wait-per-struct codegen limits.

    1. Drop Matmult waits on the PE's own completion semaphore. Tile emits
       a PE-self wait to guard PSUM write-after-write across pool-slot
       generations, but the PE drains matmuls strictly in order
       (pc-monotone completion), so a PE instruction's write never
       overtakes an earlier PE instruction's write — the self-wait is
       redundant. The cross-engine wait (DVE reader of the previous slot
       generation) is load-bearing and is kept.
    2. Reduce the kernel-tail Drain's waits to just the output-DMA
       semaphore. In this kernel's dependency chain the output DMA waits
       on all DVE work, which waits on all PE work, which waits on the
       input DMA — so output-DMA completion transitively implies every
       other wait the drain would perform.
    """
    # Find the semaphore updated by the DMA that writes the external output.
    out_sems = set()
    for fn in nc.m.functions:
        for blk in fn.blocks:
            for ins in blk.instructions:
                if type(ins).__name__ != "InstDMACopy":
                    continue
                if any(getattr(o, "memref", None) == "out" for o in ins.outs):
                    for u in ins.sync_info.on_update:
                        out_sems.add(u.ant_name)
    assert out_sems, "output DMA not found"

    n_multi = 0
    for fn in nc.m.functions:
        for blk in fn.blocks:
            for ins in blk.instructions:
                tn = type(ins).__name__
                si = ins.sync_info
                if si is None:
                    continue
                if tn == "InstMatmult":
                    waits = list(si.on_wait)
                    if any(
                        w.ant_name and w.ant_name.startswith("PE_") for w in waits
                    ):
                        si.on_wait = [
                            w
                            for w in waits
                            if not (w.ant_name and w.ant_name.startswith("PE_"))
                        ]
                        ins.sync_info = si
                    if len(ins.sync_info.on_wait) > 1:
                        n_multi += 1
                elif tn == "InstDrain" and len(si.on_wait) > 1:
                    keep = [w for w in si.on_wait if w.ant_name in out_sems]
                    assert keep, (
                        f"tail drain {ins.name} lacks an output-DMA sem wait: "
                        f"{[(w.ant_name, w.wait_value) for w in si.on_wait]}"
                    )
                    si.on_wait = keep
                    ins.sync_info = si
    assert n_multi == 0, f"{n_multi} Matmults still carry >1 sync wait"


_NC_CACHE = {}


def _get_nc():
    if "nc" not in _NC_CACHE:
        _NC_CACHE["nc"] = _build_nc()
    return _NC_CACHE["nc"]


def _make_in_maps(preds, gts):
    preds = np.ascontiguousarray(np.asarray(preds, dtype=np.float32))
    gts = np.ascontiguousarray(np.asarray(gts, dtype=np.float32))
    in_maps = []
    for b in range(B):
        x = gts[b]  # [N, 3]
        y = preds[b]  # [N, 3]
        rx = np.sum(x * x, axis=-1)  # [N]
        ry = np.sum(y * y, axis=-1)  # [N]
        ab = np.empty((D5, 2 * N), np.float32)
        ab[0:3, :N] = (-2.0 * x).T
        ab[3, :N] = rx
        ab[4, :N] = 1.0
        ab[0:3, N:] = y.T
        ab[3, N:] = 1.0
        ab[4, N:] = ry
        in_maps.append({"ab": ab})
    return in_maps


def run_device(preds, gts, **spmd_kwargs):
    """Run the on-device kernel; returns (per-core [128,1] partials, raw results)."""
    from concourse.bass_utils import run_bass_kernel_spmd

    nc = _get_nc()
    in_maps = _make_in_maps(preds, gts)
    res = run_bass_kernel_spmd(nc, in_maps, list(range(N_CORES)), **spmd_kwargs)
    partials = [np.asarray(r["out"]) for r in res.results]
    return partials, res


def kernel(preds, gts):
    partials, _ = run_device(preds, gts)
    total = np.sum(np.stack(partials, 0), dtype=np.float32)
    return np.asarray(total, dtype=np.float32)


# revision 9
# speedup vs baseline: 1.3193x; 1.3193x over previous
"""Chamfer loss kernel for Trainium2 (8 NeuronCores, batch-parallel).

Problem: preds [8, 4096, 3] f32, gts [8, 4096, 3] f32.
  P[b,n,m] = ||gts[b,n] - preds[b,m]||^2  (expanded form)
  loss = sum_{b,m} min_n P[b,n,m] + sum_{b,n} min_m P[b,n,m]

Sharding: one batch per NeuronCore (data parallel over B=8).

Device algorithm (per core, one batch):
  Host augments points to 5-dim vectors so a single K=5 matmul emits
  squared distances directly into PSUM:
      a_n = [-2*x_n, ||x_n||^2, 1]   (x = gts row)
      b_m = [ y_m,   1, ||y_m||^2]   (y = preds row)
      a_n . b_m = ||x_n - y_m||^2  (same expanded form as the reference)
  Plain fp32 matmuls stream at 1/4 rate and split 2x in codegen; fp32r
  is full-rate but ~bf16 precision (breaks the cancellation in the
  expanded form). Instead each operand is split hi/lo into two fp16
  halves and each distance tile is computed as three full-rate fp16
  matmuls accumulated in fp32 PSUM:
      a.b ~= a_hi.b_hi + a_hi.b_lo + a_lo.b_hi   (lo.lo term ~1e-5, dropped)
  Pass A: rows = n (gts) on partitions -> row-min over m -> loss_2 terms.
  Pass B: rows = m (preds) on partitions -> row-min over n -> loss_1 terms.
  VectorE min-reduces each [128, 2048] PSUM tile along the free axis,
  then folds row mins into per-partition sums [128, 1].
Host sums the 8x128 partial sums (the gather/unshard step).
"""

import sys

import numpy as np

sys.path.insert(0, "/opt/trn_rl_repo")

B = 8
N = 4096  # points per cloud (both preds and gts)
D5 = 5  # augmented dim
P = 128  # partitions
CHUNK = 2048  # psum tile free width (4 banks)
N_CORES = 8


def _build_kernel_body(ctx, tc, out_ap, ab_ap):
    import concourse.bass as bass
    from concourse import mybir

    nc = tc.nc
    f16 = mybir.dt.float16
    f32 = mybir.dt.float32
    n_rowtiles = N // P  # 32
    n_chunks = N // CHUNK  # 2
    n_banks = CHUNK // 512  # 4

    const = ctx.enter_context(tc.tile_pool(name="const", bufs=1))
    psum = ctx.enter_context(tc.tile_pool(name="psum", bufs=2, space="PSUM"))

    # One DMA for all four operand matrices [at_hi | at_lo | bt_hi | bt_lo],
    # each [5, N] fp16 -> a single DMA semaphore for the first matmuls to
    # wait on (the Matmult/LDWEIGHTS struct has one sync-wait slot).
    ab_sb = const.tile([D5, 4 * N], f16)
    nc.sync.dma_start(out=ab_sb[:], in_=ab_ap[:])
    at_hi = ab_sb[:, 0 * N : 1 * N]
    at_lo = ab_sb[:, 1 * N : 2 * N]
    bt_hi = ab_sb[:, 2 * N : 3 * N]
    bt_lo = ab_sb[:, 3 * N : 4 * N]

    # rowmins[:, pass*32+i, h] = min over chunk h's columns of row-tile i
    rowmins = const.tile([P, 2 * n_rowtiles, n_chunks], f32)

    for pass_idx, (lhs_hi, lhs_lo, rhs_hi, rhs_lo) in enumerate(
        ((at_hi, at_lo, bt_hi, bt_lo), (bt_hi, bt_lo, at_hi, at_lo))
    ):
        for i in range(n_rowtiles):
            w_hi = lhs_hi[:, i * P : (i + 1) * P]  # [5, 128]
            w_lo = lhs_lo[:, i * P : (i + 1) * P]
            for h in range(n_chunks):
                ps = psum.tile([P, CHUNK], f32, tag="ps")
                # Three-term fp16 split, accumulated per PSUM bank. Weight
                # order hi,hi,lo minimizes LDWEIGHTS swaps.
                for term, (w, rhs_src, start, stop) in enumerate(
                    ((w_hi, rhs_hi, True, False),
                     (w_hi, rhs_lo, False, False),
                     (w_lo, rhs_hi, False, True))
                ):
                    for j in range(n_banks):
                        c0 = h * CHUNK + j * 512
                        nc.tensor.matmul(
                            ps[:, j * 512 : (j + 1) * 512],
                            w,
                            rhs_src[:, c0 : c0 + 512],
                            start=start,
                            stop=stop,
                        )
                nc.vector.tensor_reduce(
                    out=rowmins[:, pass_idx * n_rowtiles + i, h : h + 1],
                    in_=ps[:],
                    axis=mybir.AxisListType.X,
                    op=mybir.AluOpType.min,
                )

    # Combine the per-chunk mins: true row min = min over the chunk axis.
    comb = const.tile([P, 2 * n_rowtiles], f32)
    nc.vector.tensor_tensor(
        out=comb[:],
        in0=rowmins[:, :, 0],
        in1=rowmins[:, :, 1],
        op=mybir.AluOpType.min,
    )
    # Per-partition sum of all row mins (both passes).
    sums = const.tile([P, 1], f32)
    nc.vector.tensor_reduce(
        out=sums[:],
        in_=comb[:],
        axis=mybir.AxisListType.X,
        op=mybir.AluOpType.add,
    )
    nc.sync.dma_start(out=out_ap[:], in_=sums[:])


def _build_nc():
    from contextlib import ExitStack

    import concourse.bass as bass
    import concourse.tile as tile
    from concourse import mybir

    nc = bass.Bass("TRN2", target_bir_lowering=False, debug=False)
    ab = nc.dram_tensor(
        "ab", [D5, 4 * N], mybir.dt.float16, kind="ExternalInput"
    ).ap()
    out = nc.dram_tensor("out", [P, 1], mybir.dt.float32, kind="ExternalOutput").ap()
    with tile.TileContext(nc) as tc, ExitStack() as ctx:
        _build_kernel_body(ctx, tc, out, ab)
    _fix_sync_waits(nc)
    return nc


def _fix_sync_waits(nc):
    """Work around walrus's one-sync-wait-per-struct codegen limits.

    1. Drop Matmult waits on the PE's own completion semaphore. Tile emits
       a PE-self wait to guard PSUM write-after-write across pool-slot
       generations, but the PE drains matmuls strictly in order
       (pc-monotone completion), so a PE instruction's write never
       overtakes an earlier PE instruction's write — the self-wait is
       redundant. The cross-engine wait (DVE reader of the previous slot
       generation) is load-bearing and is kept.
    2. Reduce the kernel-tail Drain's waits to just the output-DMA
       semaphore. In this kernel's dependency chain the output DMA waits
       on all DVE work, which waits on all PE work, which waits on the
       input DMA — so output-DMA completion transitively implies every
       other wait the drain would perform.
    """
    # Find the semaphore updated by the DMA that writes the external output.
    out_sems = set()
    for fn in nc.m.functions:
        for blk in fn.blocks:
            for ins in blk.instructions:
                if type(ins).__name__ != "InstDMACopy":
                    continue
                if any(getattr(o, "memref", None) == "out" for o in ins.outs):
                    for u in ins.sync_info.on_update:
                        out_sems.add(u.ant_name)
    assert out_sems, "output DMA not found"

    n_multi = 0
    for fn in nc.m.functions:
        for blk in fn.blocks:
            for ins in blk.instructions:
                tn = type(ins).__name__
                si = ins.sync_info
                if si is None:
                    continue
                if tn == "InstMatmult":
                    waits = list(si.on_wait)
                    if any(
                        w.ant_name and w.ant_name.startswith("PE_") for w in waits
                    ):
                        si.on_wait = [
                            w
                            for w in waits
                            if not (w.ant_name and w.ant_name.startswith("PE_"))
                        ]
                        ins.sync_info = si
                    if len(ins.sync_info.on_wait) > 1:
                        n_multi += 1
                elif tn == "InstDrain" and len(si.on_wait) > 1:
                    keep = [w for w in si.on_wait if w.ant_name in out_sems]
                    assert keep, (
                        f"tail drain {ins.name} lacks an output-DMA sem wait: "
                        f"{[(w.ant_name, w.wait_value) for w in si.on_wait]}"
                    )
                    si.on_wait = keep
                    ins.sync_info = si
    assert n_multi == 0, f"{n_multi} Matmults still carry >1 sync wait"


_NC_CACHE = {}


def _get_nc():
    if "nc" not in _NC_CACHE:
        _NC_CACHE["nc"] = _build_nc()
    return _NC_CACHE["nc"]


def _split_f16(a):
    """Split fp32 array into (hi, lo) fp16 halves with a ~= hi + lo."""
    hi = a.astype(np.float16)
    lo = (a - hi.astype(np.float32)).astype(np.float16)
    return hi, lo


def _make_in_maps(preds, gts):
    preds = np.ascontiguousarray(np.asarray(preds, dtype=np.float32))
    gts = np.ascontiguousarray(np.asarray(gts, dtype=np.float32))
    in_maps = []
    for b in range(B):
        x = gts[b]  # [N, 3]
        y = preds[b]  # [N, 3]
        rx = np.sum(x * x, axis=-1)  # [N]
        ry = np.sum(y * y, axis=-1)  # [N]
        at = np.empty((D5, N), np.float32)
        at[0:3] = (-2.0 * x).T
        at[3] = rx
        at[4] = 1.0
        bt = np.empty((D5, N), np.float32)
        bt[0:3] = y.T
        bt[3] = 1.0
        bt[4] = ry
        at_hi, at_lo = _split_f16(at)
        bt_hi, bt_lo = _split_f16(bt)
        ab = np.concatenate([at_hi, at_lo, bt_hi, bt_lo], axis=1)  # [5, 4N] fp16
        in_maps.append({"ab": ab})
    return in_maps


def run_device(preds, gts, **spmd_kwargs):
    """Run the on-device kernel; returns (per-core [128,1] partials, raw results)."""
    from concourse.bass_utils import run_bass_kernel_spmd

    nc = _get_nc()
    in_maps = _make_in_maps(preds, gts)
    res = run_bass_kernel_spmd(nc, in_maps, list(range(N_CORES)), **spmd_kwargs)
    partials = [np.asarray(r["out"]) for r in res.results]
    return partials, res


def kernel(preds, gts):
    partials, _ = run_device(preds, gts)
    total = np.sum(np.stack(partials, 0), dtype=np.float32)
    return np.asarray(total, dtype=np.float32)


# revision 10
# speedup vs baseline: 2.8732x; 2.1777x over previous
"""Chamfer loss kernel for Trainium2 (8 NeuronCores, batch-parallel).

Problem: preds [8, 4096, 3] f32, gts [8, 4096, 3] f32.
  P[b,n,m] = ||gts[b,n] - preds[b,m]||^2  (expanded form)
  loss = sum_{b,m} min_n P[b,n,m] + sum_{b,n} min_m P[b,n,m]

Sharding: one batch per NeuronCore (data parallel over B=8).

Device algorithm (per core, one batch):
  Host augments points to 5-dim vectors so a single K=5 matmul emits
  squared distances directly into PSUM:
      a_n = [-2*x_n, ||x_n||^2, 1]   (x = gts row)
      b_m = [ y_m,   1, ||y_m||^2]   (y = preds row)
      a_n . b_m = ||x_n - y_m||^2  (same expanded form as the reference)
  Precision/speed: plain fp32 matmuls stream at 1/4 rate (and split 2x
  in codegen); fp32r is full rate but ~bf16 precision, which breaks the
  cancellation in the expanded form. So each operand is split hi/lo into
  fp16 halves and each distance tile is three full-rate fp16 matmuls
  accumulated in fp32 PSUM:
      a.b ~= a_hi.b_hi + a_hi.b_lo + a_lo.b_hi   (lo.lo ~1e-5, dropped)
  K=5 uses only 5 of 128 PE rows, so matmuls are packed 4x into the
  PE's 32-row tiles (tile_position (0,0)/(32,0)/(64,0)/(96,0)): four
  row-tiles i are computed concurrently, each writing its own PSUM
  bank. Row-group q reads its operands from SBUF partitions 32q..32q+4,
  so the host replicates the fp16 operand block at partition offsets
  0/32/64/96.
  Pass A: rows = n (gts) on partitions -> row-min over m -> loss_2 terms.
  Pass B: rows = m (preds) on partitions -> row-min over n -> loss_1 terms.
  VectorE min-reduces each [128, 4x512] PSUM tile along the free axis
  (keeping the four banks = four row-tiles separate), then folds row
  mins into per-partition sums [128, 1].
Host sums the 8x128 partial sums (the gather/unshard step).
"""

import sys

import numpy as np

sys.path.insert(0, "/opt/trn_rl_repo")

B = 8
N = 4096  # points per cloud (both preds and gts)
D5 = 5  # augmented dim
P = 128  # partitions
N_CORES = 8
NBANK = 4  # psum banks per tile = concurrently packed row-tiles
NGRP = N // P // NBANK  # 8 groups of 4 row-tiles
NCHUNK = N // 512  # 8 rhs chunks of 512


def _build_kernel_body(ctx, tc, out_ap, ab_ap):
    import concourse.bass as bass
    from concourse import mybir

    nc = tc.nc
    f16 = mybir.dt.float16
    f32 = mybir.dt.float32

    const = ctx.enter_context(tc.tile_pool(name="const", bufs=1))
    psum = ctx.enter_context(tc.tile_pool(name="psum", bufs=2, space="PSUM"))

    # [at_hi | at_lo | bt_hi | bt_lo], each [5, N] fp16, replicated on
    # partition groups 0/32/64/96 (one copy per PE row-tile). One DMA ->
    # a single DMA semaphore for the first matmuls to wait on (the
    # Matmult/LDWEIGHTS struct has one sync-wait slot).
    ab_sb = const.tile([P, 4 * N], f16)
    nc.sync.dma_start(out=ab_sb[:], in_=ab_ap[:])

    def blk(q, which, cols):
        # operand block `which` on partition group q, column slice `cols`
        return ab_sb[32 * q : 32 * q + D5, which * N + cols.start : which * N + cols.stop]

    AT_HI, AT_LO, BT_HI, BT_LO = 0, 1, 2, 3

    # rowmins[:, (pass*NGRP+g)*NBANK+q, c] = min over chunk c's columns of
    # row-tile i = g*NBANK+q
    rowmins = const.tile([P, 2 * NGRP * NBANK, NCHUNK], f32)

    for pass_idx, (lhs_hi, lhs_lo, rhs_hi, rhs_lo) in enumerate(
        ((AT_HI, AT_LO, BT_HI, BT_LO), (BT_HI, BT_LO, AT_HI, AT_LO))
    ):
        for g in range(NGRP):
            for c in range(NCHUNK):
                cols = slice(c * 512, (c + 1) * 512)
                ps = psum.tile([P, NBANK * 512], f32, tag="ps")
                # Three fp16 split terms, each a wave of 4 row-group-packed
                # matmuls (concurrent in the PE array, separate PSUM banks).
                for w_which, r_which, start, stop in (
                    (lhs_hi, rhs_hi, True, False),
                    (lhs_hi, rhs_lo, False, False),
                    (lhs_lo, rhs_hi, False, True),
                ):
                    for q in range(NBANK):
                        i = g * NBANK + q
                        w = blk(q, w_which, slice(i * P, (i + 1) * P))  # [5,128]
                        r = blk(q, r_which, cols)  # [5,512]
                        nc.tensor.matmul(
                            ps[:, q * 512 : (q + 1) * 512],
                            w,
                            r,
                            start=start,
                            stop=stop,
                            tile_position=(32 * q, 0),
                        )
                # Min over the 512 columns of each bank separately:
                # [128, 4, 512] -> [128, 4], written per-chunk (stride NCHUNK).
                base = (pass_idx * NGRP + g) * NBANK
                nc.vector.tensor_reduce(
                    out=rowmins[:, base : base + NBANK, c : c + 1],
                    in_=ps[:].rearrange("p (q f) -> p q f", q=NBANK),
                    axis=mybir.AxisListType.X,
                    op=mybir.AluOpType.min,
                )

    # True row min per row-tile = min over the NCHUNK chunk-mins.
    comb = const.tile([P, 2 * NGRP * NBANK], f32)
    nc.vector.tensor_reduce(
        out=comb[:],
        in_=rowmins[:],
        axis=mybir.AxisListType.X,
        op=mybir.AluOpType.min,
    )
    # Per-partition sum of all row mins (both passes).
    sums = const.tile([P, 1], f32)
    nc.vector.tensor_reduce(
        out=sums[:],
        in_=comb[:],
        axis=mybir.AxisListType.X,
        op=mybir.AluOpType.add,
    )
    nc.sync.dma_start(out=out_ap[:], in_=sums[:])


def _build_nc():
    from contextlib import ExitStack

    import concourse.bass as bass
    import concourse.tile as tile
    from concourse import mybir

    nc = bass.Bass("TRN2", target_bir_lowering=False, debug=False)
    ab = nc.dram_tensor(
        "ab", [P, 4 * N], mybir.dt.float16, kind="ExternalInput"
    ).ap()
    out = nc.dram_tensor("out", [P, 1], mybir.dt.float32, kind="ExternalOutput").ap()
    with tile.TileContext(nc) as tc, ExitStack() as ctx:
        _build_kernel_body(ctx, tc, out, ab)
    _fix_sync_waits(nc)
    return nc


def _fix_sync_waits(nc):
    """Work around walrus's one-sync-wait-per-struct codegen limits.

    1. Drop Matmult waits on the PE's own completion semaphore. Tile emits
       a PE-self wait to guard PSUM write-after-write across pool-slot
       generations, but the PE drains matmuls strictly in order
       (pc-monotone completion), so a PE instruction's write never
       overtakes an earlier PE instruction's write — the self-wait is
       redundant. The cross-engine wait (DVE reader of the previous slot
       generation) is load-bearing and is kept.
    2. Reduce the kernel-tail Drain's waits to just the output-DMA
       semaphore. In this kernel's dependency chain the output DMA waits
       on all DVE work, which waits on all PE work, which waits on the
       input DMA — so output-DMA completion transitively implies every
       other wait the drain would perform.
    """
    # Find the semaphore updated by the DMA that writes the external output.
    out_sems = set()
    for fn in nc.m.functions:
        for blk in fn.blocks:
            for ins in blk.instructions:
                if type(ins).__name__ != "InstDMACopy":
                    continue
                if any(getattr(o, "memref", None) == "out" for o in ins.outs):
                    for u in ins.sync_info.on_update:
                        out_sems.add(u.ant_name)
    assert out_sems, "output DMA not found"

    n_multi = 0
    for fn in nc.m.functions:
        for blk in fn.blocks:
            for ins in blk.instructions:
                tn = type(ins).__name__
                si = ins.sync_info
                if si is None:
                    continue
                if tn == "InstMatmult":
                    waits = list(si.on_wait)
                    if any(
                        w.ant_name and w.ant_name.startswith("PE_") for w in waits
                    ):
                        si.on_wait = [
                            w
                            for w in waits
                            if not (w.ant_name and w.ant_name.startswith("PE_"))
                        ]
                        ins.sync_info = si
                    if len(ins.sync_info.on_wait) > 1:
                        n_multi += 1
                elif tn == "InstDrain" and len(si.on_wait) > 1:
                    keep = [w for w in si.on_wait if w.ant_name in out_sems]
                    assert keep, (
                        f"tail drain {ins.name} lacks an output-DMA sem wait: "
                        f"{[(w.ant_name, w.wait_value) for w in si.on_wait]}"
                    )
                    si.on_wait = keep
                    ins.sync_info = si
    assert n_multi == 0, f"{n_multi} Matmults still carry >1 sync wait"


_NC_CACHE = {}


def _get_nc():
    if "nc" not in _NC_CACHE:
        _NC_CACHE["nc"] = _build_nc()
    return _NC_CACHE["nc"]


def _split_f16(a):
    """Split fp32 array into (hi, lo) fp16 halves with a ~= hi + lo."""
    hi = a.astype(np.float16)
    lo = (a - hi.astype(np.float32)).astype(np.float16)
    return hi, lo


def _make_in_maps(preds, gts):
    preds = np.ascontiguousarray(np.asarray(preds, dtype=np.float32))
    gts = np.ascontiguousarray(np.asarray(gts, dtype=np.float32))
    in_maps = []
    for b in range(B):
        x = gts[b]  # [N, 3]
        y = preds[b]  # [N, 3]
        rx = np.sum(x * x, axis=-1)  # [N]
        ry = np.sum(y * y, axis=-1)  # [N]
        at = np.empty((D5, N), np.float32)
        at[0:3] = (-2.0 * x).T
        at[3] = rx
        at[4] = 1.0
        bt = np.empty((D5, N), np.float32)
        bt[0:3] = y.T
        bt[3] = 1.0
        bt[4] = ry
        at_hi, at_lo = _split_f16(at)
        bt_hi, bt_lo = _split_f16(bt)
        block = np.concatenate([at_hi, at_lo, bt_hi, bt_lo], axis=1)  # [5, 4N]
        ab = np.zeros((P, 4 * N), np.float16)
        for q in range(NBANK):  # replicate for each PE row-tile group
            ab[32 * q : 32 * q + D5] = block
        in_maps.append({"ab": ab})
    return in_maps


def run_device(preds, gts, **spmd_kwargs):
    """Run the on-device kernel; returns (per-core [128,1] partials, raw results)."""
    from concourse.bass_utils import run_bass_kernel_spmd

    nc = _get_nc()
    in_maps = _make_in_maps(preds, gts)
    res = run_bass_kernel_spmd(nc, in_maps, list(range(N_CORES)), **spmd_kwargs)
    partials = [np.asarray(r["out"]) for r in res.results]
    return partials, res


def kernel(preds, gts):
    partials, _ = run_device(preds, gts)
    total = np.sum(np.stack(partials, 0), dtype=np.float32)
    return np.asarray(total, dtype=np.float32)
